# revision 1
# baseline (speedup 1.0000x reference)
"""Trainium2 Bass kernel for nn_ContextEmbedding (embedding lookup + masked MLP branches).

Strategy (data-parallel over 8 cores, batch-sharded):
  out[r, :] = onehot8(tok[r]) @ special_table            (~all rows; exact, incl. zeros)
            + [tok[r]==CLS]  * relu(LN(x3 @ cls_w + b))  (~1/76 of rows)
            + [tok[r]==CTX]  * relu(LN(x16 @ ctx_w + b)) (~1/76 of rows)

Dense pass: one bf16 matmul per 128-row chunk (one-hot is exact in bf16; the fp32
table is split into bf16 hi+lo halves stacked along K so a single K=16 matmul
reconstructs ~fp32 precision), then PSUM->SBUF copy and a contiguous DMA out.

Sparse fixup: the CLS/CTX rows are compacted on host, the branch MLP+LN+ReLU runs
on a handful of 128-row tiles, the per-row results (plus the token's table row)
are scattered back over the dense output via indirect DMA (padding lanes carry
out-of-bounds indices and are skipped).
"""

import os
import time
import numpy as np
import ml_dtypes

import concourse.bass as bass
import concourse.bacc as bacc
import concourse.mybir as mybir
from concourse.tile import TileContext
from concourse.bass_utils import run_bass_kernel_spmd

N_CORES = 8
B, S, D = 256, 512, 256
NUM_CONTEXT = 16
NUM_SPECIAL = 8
SPECIAL_OFFSET = 68  # 52 cards + 16 bet bins
CLS_TOK = SPECIAL_OFFSET + 0
CTX_TOK = SPECIAL_OFFSET + 1
LN_EPS = 1e-5
P = 128
R = (B * S) // N_CORES       # rows per core
CHUNKS = R // P
OOB_PAD = 1 << 20

_LAST = {}


def _branch_host(W, bvec, g, b_ln):
    """Host-side prep of one MLP branch: center the linear layer so the LN mean
    subtraction folds into the weights, and (when LN gamma is uniform) fold
    gamma in too. Returns the rhs matrix for the device matmul plus the scale
    constant for the sqrt(var+eps) activation."""
    W64 = np.asarray(W, np.float64)
    b64 = np.asarray(bvec, np.float64)
    g64 = np.asarray(g, np.float64)
    bln64 = np.asarray(b_ln, np.float64)
    wm = W64.mean(axis=1, keepdims=True)
    bm = b64.mean()
    Wc = W64 - wm
    bc = b64 - bm
    g_uniform = bool(np.all(g64 == g64.flat[0]))
    use_bln = bool(np.any(bln64 != 0.0))
    if g_uniform and not use_bln:
        gv = float(g64.flat[0])
        if gv == 0.0:
            return dict(mode="zero")
        rhs = np.concatenate([Wc * gv, (bc * gv)[None, :]], axis=0)  # [K+1, D]
        return dict(mode="fast", rhs=rhs.astype(np.float32),
                    sqrt_scale=float(1.0 / (D * gv * gv)))
    # general: rhs = [hc block | hg block]; hc drives the variance, hg the output
    Wg = Wc * g64[None, :]
    bg = bc * g64
    rhs = np.concatenate(
        [np.concatenate([Wc, bc[None, :]], axis=0),
         np.concatenate([Wg, bg[None, :]], axis=0)], axis=1)  # [K+1, 2D]
    return dict(mode="general", rhs=rhs.astype(np.float32),
                sqrt_scale=float(1.0 / D), use_bln=use_bln,
                bln_rep=np.tile(bln64.astype(np.float32)[None, :], (P, 1)))


def _compact(tok_flat, x_flat, token_value, k_feats):
    """Compact the rows with token==token_value, per core. Returns per-core
    transposed (bias-augmented) feature tiles and int32 scatter indices padded
    with OOB_PAD (skipped by the indirect DMA bounds check)."""
    per_core = [np.nonzero(tok_flat[c * R:(c + 1) * R] == token_value)[0]
                for c in range(N_CORES)]
    max_n = max(len(rows) for rows in per_core)
    if max_n == 0:
        return None
    T = (max_n + P - 1) // P
    npad = T * P
    xgts, idxs = [], []
    for c in range(N_CORES):
        rows = per_core[c]
        n = len(rows)
        xg = np.zeros((k_feats + 1, npad), np.float32)
        xg[k_feats, :] = 1.0  # bias row
        if n:
            xs = x_flat[c * R + rows][:, :k_feats]
            xg[:k_feats, :n] = np.ascontiguousarray(xs.T.astype(np.float32))
        idx = np.full((P, T), OOB_PAD, np.int32)
        if n:
            jj = np.arange(n)
            idx[jj % P, jj // P] = rows.astype(np.int32)
        xgts.append(np.ascontiguousarray(xg))
        idxs.append(np.ascontiguousarray(idx))
    return dict(T=T, xgt=xgts, idx=idxs)


def _build(meta):
    nc = bacc.Bacc(None)
    f32 = mybir.dt.float32
    bf16 = mybir.dt.bfloat16
    i32 = mybir.dt.int32
    Relu = mybir.ActivationFunctionType.Relu
    Sqrt = mybir.ActivationFunctionType.Sqrt
    Square = mybir.ActivationFunctionType.Square

    oh_d = nc.dram_tensor("oh", [2 * NUM_SPECIAL, R], bf16, kind="ExternalInput")
    rhs16_d = nc.dram_tensor("rhs16", [2 * NUM_SPECIAL, D], bf16, kind="ExternalInput")
    out_d = nc.dram_tensor("out", [R, D], f32, kind="ExternalOutput")

    br_handles = {}
    for name, br in meta["branches"].items():
        if br is None:
            continue
        K1, ND = br["host"]["rhs"].shape
        T = br["comp"]["T"]
        h = dict(
            xgt=nc.dram_tensor(f"xgt_{name}", [K1, T * P], f32, kind="ExternalInput"),
            w=nc.dram_tensor(f"w_{name}", [K1, ND], f32, kind="ExternalInput"),
            idx=nc.dram_tensor(f"idx_{name}", [P, T], i32, kind="ExternalInput"),
            tbl=nc.dram_tensor(f"tbl_{name}", [P, D], f32, kind="ExternalInput"),
        )
        if br["host"].get("use_bln"):
            h["bln"] = nc.dram_tensor(f"bln_{name}", [P, D], f32, kind="ExternalInput")
        br_handles[name] = h

    with TileContext(nc) as tc:
        with (
            tc.tile_pool(name="const", bufs=1) as cpool,
            tc.tile_pool(name="dense_in", bufs=6) as inpool,
            tc.tile_pool(name="dense_out", bufs=8) as outpool,
            tc.tile_pool(name="fix", bufs=max(4, 2 * meta["t_max"])) as fixpool,
        ):
            rhs16_sb = cpool.tile([2 * NUM_SPECIAL, D], bf16, tag="rhs16")
            nc.sync.dma_start(out=rhs16_sb[:], in_=rhs16_d[:])
            eps_sb = cpool.tile([P, 1], f32, tag="eps")
            nc.vector.memset(eps_sb[:], LN_EPS)

            br_sb = {}
            for name, h in br_handles.items():
                br = meta["branches"][name]
                K1, ND = br["host"]["rhs"].shape
                T = br["comp"]["T"]
                d = {}
                d["xgt"] = cpool.tile([K1, T * P], f32, tag=f"xgt_{name}", name=f"xgt_{name}_sb")
                nc.sync.dma_start(out=d["xgt"][:], in_=h["xgt"][:])
                d["w"] = cpool.tile([K1, ND], f32, tag=f"w_{name}", name=f"w_{name}_sb")
                nc.sync.dma_start(out=d["w"][:], in_=h["w"][:])
                d["idx"] = cpool.tile([P, T], i32, tag=f"idx_{name}", name=f"idx_{name}_sb")
                nc.sync.dma_start(out=d["idx"][:], in_=h["idx"][:])
                d["tbl"] = cpool.tile([P, D], f32, tag=f"tbl_{name}", name=f"tbl_{name}_sb")
                nc.sync.dma_start(out=d["tbl"][:], in_=h["tbl"][:])
                if "bln" in h:
                    d["bln"] = cpool.tile([P, D], f32, tag=f"bln_{name}", name=f"bln_{name}_sb")
                    nc.sync.dma_start(out=d["bln"][:], in_=h["bln"][:])
                br_sb[name] = d

            # ---- dense pass ----
            # G row-chunks per group: one SP-ring load, G matmuls, one big ACT
            # copy, one ACT-ring store. Fresh tiles every group + same-engine
            # (ACT) copy->store keep every HWDGE DMA at <=1 sync wait (the
            # hardware instruction only fits one wait + its completion update).
            G = 4
            NG = CHUNKS // G
            out_v = out_d[:].rearrange("(g q p) d -> g p q d", p=P, q=G)
            with tc.tile_pool(name="psd", bufs=4, space="PSUM") as psd:
                for g in range(NG):
                    oh_t = inpool.tile([2 * NUM_SPECIAL, G * P], bf16, tag="oh")
                    nc.sync.dma_start(out=oh_t[:], in_=oh_d[:, g * G * P:(g + 1) * G * P])
                    ps = psd.tile([P, G * D], f32, tag="dps")
                    for q in range(G):
                        nc.tensor.matmul(out=ps[:, q * D:(q + 1) * D],
                                         lhsT=oh_t[:, q * P:(q + 1) * P],
                                         rhs=rhs16_sb[:], start=True, stop=True)
                    ot = outpool.tile([P, G * D], f32, tag="dout")
                    if g % 2 == 0:
                        nc.vector.tensor_copy(out=ot[:], in_=ps[:])
                        nc.sync.dma_start(out=out_v[g], in_=ot[:])
                    else:
                        nc.scalar.copy(out=ot[:], in_=ps[:])
                        nc.scalar.dma_start(out=out_v[g], in_=ot[:])

            # ---- sparse fixup (both branches interleaved, func-major to
            # minimize ACT LUT-table swaps) ----
            pairs = []
            if not os.environ.get("KERNEL_DENSE_ONLY"):
                for name in ("ctx", "cls"):
                    if name in br_sb:
                        pairs.extend((name, t) for t in range(meta["branches"][name]["comp"]["T"]))
            if pairs:
                with tc.tile_pool(name="psf", bufs=meta["t_max"], space="PSUM") as psf:
                    psA, ss, sd, rstd, rr = {}, {}, {}, {}, {}
                    for name, t in pairs:
                        br = meta["branches"][name]
                        ND = br["host"]["rhs"].shape[1]
                        s = br_sb[name]
                        psA[(name, t)] = psf.tile([P, ND], f32, tag=f"psA_{name}", name=f"psA_{name}_{t}")
                        nc.tensor.matmul(out=psA[(name, t)][:],
                                         lhsT=s["xgt"][:, t * P:(t + 1) * P],
                                         rhs=s["w"][:], start=True, stop=True)
                    for name, t in pairs:
                        sq = fixpool.tile([P, D], f32, tag="sq")
                        ss[(name, t)] = fixpool.tile([P, 1], f32, tag="ss", name=f"ss_{name}_{t}")
                        nc.scalar.activation(out=sq[:], in_=psA[(name, t)][:, 0:D],
                                             func=Square, accum_out=ss[(name, t)][:])
                    for name, t in pairs:
                        sqs = meta["branches"][name]["host"]["sqrt_scale"]
                        sd[(name, t)] = fixpool.tile([P, 1], f32, tag="sd", name=f"sd_{name}_{t}")
                        nc.scalar.activation(out=sd[(name, t)][:], in_=ss[(name, t)][:],
                                             func=Sqrt, bias=eps_sb[:, 0:1], scale=sqs)
                    for name, t in pairs:
                        rstd[(name, t)] = fixpool.tile([P, 1], f32, tag="rstd", name=f"rstd_{name}_{t}")
                        nc.vector.reciprocal(out=rstd[(name, t)][:], in_=sd[(name, t)][:])
                    for name, t in pairs:
                        host = meta["branches"][name]["host"]
                        rr[(name, t)] = fixpool.tile([P, D], f32, tag="rr", name=f"rr_{name}_{t}")
                        if host["mode"] == "fast":
                            nc.scalar.activation(out=rr[(name, t)][:],
                                                 in_=psA[(name, t)][:, 0:D],
                                                 func=Relu, scale=rstd[(name, t)][:, 0:1])
                        else:
                            pre = fixpool.tile([P, D], f32, tag="pre")
                            nc.vector.tensor_scalar_mul(out=pre[:],
                                                        in0=psA[(name, t)][:, D:2 * D],
                                                        scalar1=rstd[(name, t)][:, 0:1])
                            if host.get("use_bln"):
                                nc.vector.tensor_add(out=pre[:], in0=pre[:],
                                                     in1=br_sb[name]["bln"][:])
                            nc.scalar.activation(out=rr[(name, t)][:], in_=pre[:],
                                                 func=Relu)
                    for name, t in pairs:
                        s = br_sb[name]
                        fx = fixpool.tile([P, D], f32, tag="fx")
                        nc.vector.tensor_add(out=fx[:], in0=rr[(name, t)][:],
                                             in1=s["tbl"][:])
                        nc.gpsimd.indirect_dma_start(
                            out=out_d[:],
                            out_offset=bass.IndirectOffsetOnAxis(
                                ap=s["idx"][:, t:t + 1], axis=0),
                            in_=fx[:],
                            in_offset=None,
                            bounds_check=R - 1,
                            oob_is_err=False,
                        )
    nc.compile()
    return nc


def kernel(**inputs):
    tok = np.asarray(inputs["token_ids"]).reshape(-1).astype(np.int64)
    x = np.asarray(inputs["context_features"], np.float32).reshape(-1, NUM_CONTEXT)
    st = np.asarray(inputs["special_table"], np.float32)

    # one-hot over the 8 special ids, exact in bf16; stacked twice for the
    # hi/lo split of the fp32 table (single K=16 bf16 matmul ~ fp32 result)
    oh8 = (tok[None, :] == (SPECIAL_OFFSET + np.arange(NUM_SPECIAL))[:, None])
    oh16 = np.concatenate([oh8, oh8], axis=0).astype(ml_dtypes.bfloat16)
    t_hi = st.astype(ml_dtypes.bfloat16)
    t_lo = (st - t_hi.astype(np.float32)).astype(ml_dtypes.bfloat16)
    rhs16 = np.ascontiguousarray(np.concatenate([t_hi, t_lo], axis=0))

    branches = {}
    comp_cls = _compact(tok, x, CLS_TOK, 3)
    comp_ctx = _compact(tok, x, CTX_TOK, NUM_CONTEXT)
    host_cls = _branch_host(inputs["cls_w"], inputs["cls_b"],
                            inputs["cls_ln_g"], inputs["cls_ln_b"])
    host_ctx = _branch_host(inputs["ctx_w"], inputs["ctx_b"],
                            inputs["ctx_ln_g"], inputs["ctx_ln_b"])
    branches["cls"] = (dict(host=host_cls, comp=comp_cls)
                       if comp_cls is not None and host_cls["mode"] != "zero" else None)
    branches["ctx"] = (dict(host=host_ctx, comp=comp_ctx)
                       if comp_ctx is not None and host_ctx["mode"] != "zero" else None)
    t_max = max([br["comp"]["T"] for br in branches.values() if br] + [1])
    meta = dict(branches=branches, t_max=t_max)

    nc = _build(meta)

    in_maps = []
    for c in range(N_CORES):
        m = {
            "oh": np.ascontiguousarray(oh16[:, c * R:(c + 1) * R]),
            "rhs16": rhs16,
        }
        for name, tbl_row in (("cls", 0), ("ctx", 1)):
            br = branches[name]
            if br is None:
                continue
            m[f"xgt_{name}"] = br["comp"]["xgt"][c]
            m[f"w_{name}"] = np.ascontiguousarray(br["host"]["rhs"])
            m[f"idx_{name}"] = br["comp"]["idx"][c]
            m[f"tbl_{name}"] = np.ascontiguousarray(
                np.tile(st[tbl_row][None, :], (P, 1)))
            if br["host"].get("use_bln"):
                m[f"bln_{name}"] = br["host"]["bln_rep"]
        in_maps.append(m)

    res = None
    for attempt in range(3):
        try:
            res = run_bass_kernel_spmd(nc, in_maps, core_ids=list(range(N_CORES)))
            break
        except Exception:
            # transient device errors (e.g. NRT unit-unrecoverable) usually
            # clear after a pause; rebuild the program so no stale executable
            # state is reused
            if attempt == 2:
                raise
            time.sleep(10)
            nc = _build(meta)
    _LAST["results"] = res
    _LAST["meta"] = meta

    out = np.concatenate(
        [res.results[c]["out"].reshape(B // N_CORES, S, D) for c in range(N_CORES)],
        axis=0)
    return np.ascontiguousarray(out.astype(np.float32))

